# revision 17
# baseline (speedup 1.0000x reference)
"""Trainium2 Bass kernel for nn_Block_Spatial_Battleneck_Share_spatial_reduction.

Data-parallel over batch: 32 images -> 8 cores x 4 images.
Layout: activations [channel_partition, spatial_free]; bf16 compute, fp32 PSUM.
"""

import numpy as np
import ml_dtypes

import concourse.bass as bass
import concourse.mybir as mybir
import concourse.tile as tile
import bass_rust
from concourse.vector_clock import ScopedClock
from concourse.bass_utils import run_bass_kernel_spmd

# ---- problem constants (hardcoded) ----
B, C, H, W = 32, 384, 32, 32
N = H * W                 # 1024
KD, NH = 32, 8
MQ = KD * NH              # 256
HID = 4 * C               # 1536
CORES = 8
BPC = B // CORES          # 4 images per core
NC_C = C // 128           # 3 channel tiles
NC_Q = MQ // 128          # 2 q tiles
NC_H = HID // 128         # 12 hidden tiles
NCHUNK = N // 512         # 2 matmul column chunks

DT = mybir.dt.bfloat16
F32 = mybir.dt.float32
BF16_NP = ml_dtypes.bfloat16

# tap order: center first (covers every output element -> start=True)
TAPS = [(0, 0), (-1, -1), (-1, 0), (-1, 1), (0, -1), (0, 1), (1, -1), (1, 0), (1, 1)]

# engine assignment per hidden channel-tile for the depthwise conv
# entries: 'pe' | 'dve' | 'act' | 'gps'
ASSIGN = ['pe', 'gps', 'act', 'dve', 'dve', 'gps', 'act', 'dve', 'pe', 'dve', 'gps', 'pe']

PE_TILES = [h for h, a in enumerate(ASSIGN) if a == 'pe']
DIAG_IDX = {h: i for i, h in enumerate(PE_TILES)}  # (h) -> base of 9 diag slots
NDIAG = 9 * len(PE_TILES)

# pp (per-partition f32 constants) column map
COL_ZERO = 0
COL_BQ = 1            # 2 cols
COL_BKV = 3           # 1 col (bkv tiled 4x)
COL_BP = 4            # 3 cols
COL_B1 = 7            # 12 cols
COL_B2 = 19           # 3 cols
COL_BDW = 22          # 12 cols
COL_WT = 34           # 108 cols: (h, tap) -> COL_WT + 9*h + tap
COL_WTN = COL_WT + 9 * NC_H   # negated tap weights (x-wrap corrections)
NCOL = COL_WTN + 9 * NC_H


def _split_drain_and_barrier(self, tick_clock, wait_clock):
    # walrus in this env only accepts one sync-wait per CTRL(Drain): split.
    drain_inst = self.nc.sync.drain()
    wait_clock.add_sem_waits(drain_inst.ins, ScopedClock({None: tick_clock.global_clock}))
    si0 = drain_inst.ins.sync_info
    waits = list(si0.on_wait) if si0 is not None else []
    if len(waits) > 1:
        drain_inst.ins.sync_info = bass_rust.SyncInfo(on_wait=waits[:1], on_update=[])
        for i in range(1, len(waits)):
            extra = self.nc.sync.drain()
            extra.ins.sync_info = bass_rust.SyncInfo(on_wait=waits[i:i + 1], on_update=[])
    self.nc.all_engine_barrier()
    popped = self.nc._tile_sem_poison_stack.pop()
    assert popped is self._sem_poison
    self.nc.clear_and_free_semaphores(list(self.sems.allocated().values()))
    self.nc.all_engine_barrier()


tile.TileContext._drain_and_barrier = _split_drain_and_barrier


def _bcast_ap(sl, count):
    """Leading partition-broadcast dim (step 0) on an AP slice, for DMA."""
    return bass.AP(tensor=sl.tensor, offset=sl.offset, ap=[[0, count]] + list(sl.ap))


def tap_ranges(dy, dx):
    """(out_y, out_x, in_y, in_x) slices for a shifted tap with zero padding."""
    y0, y1 = max(0, -dy), H - max(0, dy)
    x0, x1 = max(0, -dx), W - max(0, dx)
    return (y0, y1, x0, x1)


def build_nc(iters=1):
    nc = bass.Bass(debug=False)

    x_e = nc.declare_dram_parameter("x", [BPC, C, N], DT, isOutput=False)
    wqT_e = nc.declare_dram_parameter("wqT", [C, MQ], DT, isOutput=False)
    wkvT_e = nc.declare_dram_parameter("wkvT", [C, KD], DT, isOutput=False)
    wpT_e = nc.declare_dram_parameter("wpT", [MQ, C], DT, isOutput=False)
    w1T_e = nc.declare_dram_parameter("w1T", [C, HID], DT, isOutput=False)
    w2T_e = nc.declare_dram_parameter("w2T", [HID, C], DT, isOutput=False)
    pp_e = nc.declare_dram_parameter("pp", [128, NCOL], F32, isOutput=False)
    sel_e = nc.declare_dram_parameter("sel", [2, 128, 8], DT, isOutput=False)
    selb_e = nc.declare_dram_parameter("selb", [2, 8, 128], DT, isOutput=False)
    ident_e = nc.declare_dram_parameter("ident", [128, 128], DT, isOutput=False)
    if NDIAG:
        diag_e = nc.declare_dram_parameter("diag", [NDIAG, 128, 128], DT, isOutput=False)
    y_e = nc.declare_dram_parameter("y", [BPC, C, N], F32, isOutput=True)

    with tile.TileContext(nc) as tc:
        with (
            tc.tile_pool(name="wp", bufs=1) as wp,
            tc.tile_pool(name="xp", bufs=BPC) as xp,
            tc.tile_pool(name="x1p", bufs=BPC) as x1p,
            tc.tile_pool(name="kvp", bufs=1) as kvp,
            tc.tile_pool(name="trp", bufs=4) as trp,
            tc.tile_pool(name="ctxp", bufs=BPC) as ctxp,
            tc.tile_pool(name="eqp", bufs=4) as eqp,
            tc.tile_pool(name="h1p", bufs=3) as h1p,
            tc.tile_pool(name="h2p", bufs=NC_H + 1) as h2p,
            tc.tile_pool(name="accp", bufs=3) as accp,
            tc.tile_pool(name="tmpp", bufs=3) as tmpp,
            tc.tile_pool(name="yp", bufs=3) as yp,
            tc.tile_pool(name="smallp", bufs=4) as smallp,
            tc.tile_pool(name="ps_big", bufs=3, space="PSUM") as ps_big,
            tc.tile_pool(name="ps_small", bufs=2, space="PSUM") as ps_small,
        ):
            # ---------- weights into SBUF (outside the timing loop) ----------
            wq_sb = wp.tile([128, NC_C, MQ], DT)
            wkv_sb = wp.tile([128, NC_C, KD], DT)
            wpT_sb = wp.tile([128, NC_Q, C], DT)
            w1_sb = wp.tile([128, NC_C, HID], DT)
            w2_sb = wp.tile([128, NC_H, C], DT)
            pp_sb = wp.tile([128, NCOL], F32)
            sel_sb = wp.tile([128, 2, 8], DT)
            selb_sb = wp.tile([8, 2, 128], DT)
            id_sb = wp.tile([128, 128], DT)
            for c in range(NC_C):
                nc.sync.dma_start(out=wq_sb[:, c, :], in_=wqT_e[128 * c:128 * (c + 1), :])
                nc.sync.dma_start(out=wkv_sb[:, c, :], in_=wkvT_e[128 * c:128 * (c + 1), :])
                nc.sync.dma_start(out=w1_sb[:, c, :], in_=w1T_e[128 * c:128 * (c + 1), :])
            for t in range(NC_Q):
                nc.sync.dma_start(out=wpT_sb[:, t, :], in_=wpT_e[128 * t:128 * (t + 1), :])
            for h in range(NC_H):
                nc.sync.dma_start(out=w2_sb[:, h, :], in_=w2T_e[128 * h:128 * (h + 1), :])
            nc.sync.dma_start(out=pp_sb[:], in_=pp_e[:])
            nc.sync.dma_start(out=sel_sb[:], in_=sel_e.rearrange("a p b -> p a b"))
            nc.sync.dma_start(out=selb_sb[:], in_=selb_e.rearrange("a p b -> p a b"))
            nc.sync.dma_start(out=id_sb[:], in_=ident_e[:])
            if NDIAG:
                dg_sb = wp.tile([128, NDIAG, 128], DT)
                nc.sync.dma_start(out=dg_sb[:], in_=diag_e.rearrange("a p b -> p a b"))

            x_sb = [xp.tile([128, NC_C, N], DT, tag="x", name=f"x_sb{_i}") for _i in range(BPC)]
            for i in range(BPC):
                for c in range(NC_C):
                    nc.sync.dma_start(out=x_sb[i][:, c, :], in_=x_e[i, 128 * c:128 * (c + 1), :])

            def ppc(col, p0=0, p1=128):
                return pp_sb[p0:p1, col:col + 1]

            def body(_iv=None):
                # =================== phase A: kv / context (4-image packed) ===================
                kv_all = kvp.tile([128, N], DT, tag="kv")
                e_all = kvp.tile([128, N], DT, tag="e")
                sum_e = smallp.tile([128, 1], F32, tag="sume")
                r_all = smallp.tile([128, 1], F32, tag="rall")

                pkv = ps_big.tile([128, N], F32, tag="big")
                for i in range(BPC):
                    for n2 in range(NCHUNK):
                        for c in range(NC_C):
                            nc.tensor.matmul(
                                pkv[32 * i:32 * (i + 1), 512 * n2:512 * (n2 + 1)],
                                lhsT=wkv_sb[:, c, :],
                                rhs=x_sb[i][:, c, 512 * n2:512 * (n2 + 1)],
                                start=(c == 0), stop=(c == NC_C - 1),
                                tile_position=(0, 32 * i))
                nc.scalar.activation(kv_all[:], pkv[:], mybir.ActivationFunctionType.Identity,
                                     bias=ppc(COL_BKV))
                nc.scalar.activation(e_all[:], kv_all[:], mybir.ActivationFunctionType.Exp,
                                     bias=ppc(COL_ZERO), accum_out=sum_e[:])
                nc.vector.reciprocal(r_all[:], sum_e[:])

                kvT_sb, eT_sb = [], []
                for src in (kv_all, e_all):
                    dst_list = kvT_sb if src is kv_all else eT_sb
                    for half in range(2):
                        dst = trp.tile([128, 512], DT, tag="tr")
                        for j in range(4):
                            jj = 4 * half + j
                            # one transpose per bank-padded psum tile: the sim's
                            # zero-region group tracking is 2KB-granular
                            ptr = ps_small.tile([128, 128], DT, tag="trp")
                            nc.tensor.transpose(
                                ptr[:], src[:, 128 * jj:128 * (jj + 1)], id_sb[:])
                            nc.scalar.copy(dst[:, 128 * j:128 * (j + 1)], ptr[:])
                        dst_list.append(dst)

                pctx = ps_big.tile([128, 128], F32, tag="big")
                for j in range(8):
                    nc.tensor.matmul(pctx[:], lhsT=eT_sb[j // 4][:, 128 * (j % 4):128 * (j % 4 + 1)],
                                     rhs=kvT_sb[j // 4][:, 128 * (j % 4):128 * (j % 4 + 1)],
                                     start=(j == 0), stop=(j == 7))
                ctx_sb = kvp.tile([128, 128], DT, tag="ctx")
                nc.scalar.activation(ctx_sb[:], pctx[:], mybir.ActivationFunctionType.Copy,
                                     scale=r_all[:])
                ctx4 = []
                for i in range(BPC):
                    c4 = ctxp.tile([128, KD], DT, tag="ctx4")
                    sl = ctx_sb[32 * i:32 * (i + 1), 32 * i:32 * (i + 1)]
                    for g in range(4):
                        nc.sync.dma_start(out=c4[32 * g:32 * (g + 1), :], in_=sl)
                    ctx4.append(c4)

                # =================== phases B + C per image ===================
                for i in range(BPC):
                    # ---- B: attention ----
                    eq_sb = []
                    for t in range(NC_Q):
                        pq = ps_big.tile([128, N], F32, tag="big")
                        for n2 in range(NCHUNK):
                            for c in range(NC_C):
                                nc.tensor.matmul(
                                    pq[:, 512 * n2:512 * (n2 + 1)],
                                    lhsT=wq_sb[:, c, 128 * t:128 * (t + 1)],
                                    rhs=x_sb[i][:, c, 512 * n2:512 * (n2 + 1)],
                                    start=(c == 0), stop=(c == NC_C - 1))
                        eq = eqp.tile([128, N], DT, tag="eq")
                        nc.scalar.activation(eq[:], pq[:], mybir.ActivationFunctionType.Exp,
                                             bias=ppc(COL_BQ + t))
                        eq_sb.append(eq)

                    pd = ps_big.tile([8, N], F32, tag="big")
                    for n2 in range(NCHUNK):
                        nc.tensor.matmul(pd[:, 512 * n2:512 * (n2 + 1)], lhsT=sel_sb[:, 0, :],
                                         rhs=eq_sb[0][:, 512 * n2:512 * (n2 + 1)],
                                         start=True, stop=False)
                        nc.tensor.matmul(pd[:, 512 * n2:512 * (n2 + 1)], lhsT=sel_sb[:, 1, :],
                                         rhs=eq_sb[1][:, 512 * n2:512 * (n2 + 1)],
                                         start=False, stop=True)
                    rd = smallp.tile([8, N], DT, tag="rd")
                    with nc.allow_low_precision("softmax denominators are O(30+), bf16 fine"):
                        nc.vector.reciprocal(rd[:], pd[:])

                    query_sb = []
                    for t in range(NC_Q):
                        pbc = ps_big.tile([128, N], F32, tag="big")
                        for n2 in range(NCHUNK):
                            nc.tensor.matmul(pbc[:, 512 * n2:512 * (n2 + 1)],
                                             lhsT=selb_sb[:, t, :],
                                             rhs=rd[:, 512 * n2:512 * (n2 + 1)],
                                             start=True, stop=True)
                        bc = tmpp.tile([128, N], DT, tag="bc")
                        nc.scalar.copy(bc[:], pbc[:])
                        q = eqp.tile([128, N], DT, tag="query")
                        nc.vector.tensor_mul(q[:], eq_sb[t][:], bc[:])
                        query_sb.append(q)

                    att_sb = []
                    for t in range(NC_Q):
                        patt = ps_big.tile([128, N], F32, tag="big")
                        for g in range(4):
                            for n2 in range(NCHUNK):
                                nc.tensor.matmul(
                                    patt[32 * g:32 * (g + 1), 512 * n2:512 * (n2 + 1)],
                                    lhsT=ctx4[i][32 * g:32 * (g + 1), :],
                                    rhs=query_sb[t][:, 512 * n2:512 * (n2 + 1)]
                                        [32 * g:32 * (g + 1), :],
                                    start=True, stop=True,
                                    tile_position=(32 * g, 32 * g))
                        att = eqp.tile([128, N], DT, tag="att")
                        nc.scalar.activation(att[:], patt[:], mybir.ActivationFunctionType.Relu,
                                             bias=ppc(COL_ZERO))
                        att_sb.append(att)

                    x1 = x1p.tile([128, NC_C, N], DT, tag="x1")
                    for m in range(NC_C):
                        pao = ps_big.tile([128, N], F32, tag="big")
                        for n2 in range(NCHUNK):
                            for t in range(NC_Q):
                                nc.tensor.matmul(
                                    pao[:, 512 * n2:512 * (n2 + 1)],
                                    lhsT=wpT_sb[:, t, 128 * m:128 * (m + 1)],
                                    rhs=att_sb[t][:, 512 * n2:512 * (n2 + 1)],
                                    start=(t == 0), stop=(t == NC_Q - 1))
                        nc.vector.scalar_tensor_tensor(
                            x1[:, m, :], pao[:], ppc(COL_BP + m), x_sb[i][:, m, :],
                            op0=mybir.AluOpType.add, op1=mybir.AluOpType.add)

                    # ---- C1: MLP up + depthwise ----
                    h2_sb = []
                    for h in range(NC_H):
                        ph = ps_big.tile([128, N], F32, tag="big")
                        for n2 in range(NCHUNK):
                            for c in range(NC_C):
                                nc.tensor.matmul(
                                    ph[:, 512 * n2:512 * (n2 + 1)],
                                    lhsT=w1_sb[:, c, 128 * h:128 * (h + 1)],
                                    rhs=x1[:, c, 512 * n2:512 * (n2 + 1)],
                                    start=(c == 0), stop=(c == NC_C - 1))
                        h1 = h1p.tile([128, N], DT, tag="h1")
                        nc.scalar.activation(h1[:], ph[:], mybir.ActivationFunctionType.Identity,
                                             bias=ppc(COL_B1 + h))
                        h1v = h1.rearrange("p (y x) -> p y x", x=W)
                        mode = ASSIGN[h]
                        h2 = h2p.tile([128, N], DT, tag="h2")
                        h2v = h2.rearrange("p (y x) -> p y x", x=W)

                        if mode == 'pe':
                            pdw = ps_big.tile([128, N], F32, tag="big")
                            pdwv = pdw.rearrange("p (y x) -> p y x", x=W)
                            # flat 1D shifted matmuls (<=512 cols, bank-aligned splits)
                            nmm = []
                            for ti, (dy, dx) in enumerate(TAPS):
                                s = W * dy + dx
                                n0 = max(W * max(0, -dy), -s)
                                n1 = min(N - W * max(0, dy), N - s)
                                for (a, b) in ((n0, min(n1, 512)), (max(n0, 512), n1)):
                                    if a < b:
                                        nmm.append([ti, s, a, b, False])
                            # stop on the last MM touching each 2KB psum region
                            seen = set()
                            for ent in reversed(nmm):
                                half = ent[2] // 512
                                if half not in seen:
                                    seen.add(half)
                                    ent[4] = True
                            last_mm = None
                            for (ti, s, a, b, st) in nmm:
                                mm = nc.tensor.matmul(
                                    pdw[:, a:b],
                                    lhsT=dg_sb[:, 9 * DIAG_IDX[h] + ti, :],
                                    rhs=h1[:, a + s:b + s],
                                    start=(ti == 0), stop=st)
                                if last_mm is not None:
                                    # disjoint ranges carry no data dep; keep the
                                    # accumulation group in program order
                                    bass_rust.add_dep_helper(
                                        mm.ins, last_mm.ins, reason="dw group order")
                                last_mm = mm
                            # x-wraparound corrections for dx != 0 taps
                            for ti, (dy, dx) in enumerate(TAPS):
                                if dx == 0:
                                    continue
                                s = W * dy + dx
                                n0 = max(W * max(0, -dy), -s)
                                n1 = min(N - W * max(0, dy), N - s)
                                xe = W - 1 if dx == 1 else 0
                                # polluted out cols: n = W*y + xe in [n0, n1)
                                ylo = -(-(n0 - xe) // W)      # ceil
                                yhi = (n1 - 1 - xe) // W + 1  # exclusive
                                if ylo >= yhi:
                                    continue
                                # wrongly-read source: h1 flat at n + s
                                ys = (W * ylo + xe + s) // W
                                srcx = 0 if dx == 1 else W - 1
                                fix = nc.vector.scalar_tensor_tensor(
                                    pdwv[:, ylo:yhi, xe:xe + 1],
                                    h1v[:, ys:ys + (yhi - ylo), srcx:srcx + 1],
                                    ppc(COL_WTN + 9 * h + ti),
                                    pdwv[:, ylo:yhi, xe:xe + 1],
                                    op0=mybir.AluOpType.mult, op1=mybir.AluOpType.add)
                                # ensure the PE accumulation group is closed first
                                bass_rust.add_dep_helper(
                                    fix.ins, last_mm.ins,
                                    reason="dw x-wrap fix after group close")
                            nc.scalar.activation(h2[:], pdw[:], mybir.ActivationFunctionType.Relu,
                                                 bias=ppc(COL_BDW + h))
                        else:
                            acc = accp.tile([128, N], DT, tag="acc")
                            accv = acc.rearrange("p (y x) -> p y x", x=W)
                            wcol = lambda ti: ppc(COL_WT + 9 * h + ti)
                            # product + add pairs; products on DVE or ACT,
                            # adds on DVE ('dve'/'act') or GPSIMD ('gps')
                            if mode == 'act':
                                nc.scalar.mul(acc[:], h1[:], wcol(0))
                            else:
                                nc.vector.tensor_scalar(acc[:], h1[:], wcol(0), None,
                                                        op0=mybir.AluOpType.mult)
                            for ti, (dy, dx) in enumerate(TAPS[1:], start=1):
                                y0, y1, x0, x1r = tap_ranges(dy, dx)
                                tmp = tmpp.tile([128, N], DT, tag="dwtmp")
                                tmpv = tmp.rearrange("p (y x) -> p y x", x=W)
                                src = h1v[:, y0 + dy:y1 + dy, x0 + dx:x1r + dx]
                                if mode == 'act':
                                    nc.scalar.mul(tmpv[:, y0:y1, x0:x1r], src, wcol(ti))
                                else:
                                    nc.vector.tensor_scalar(tmpv[:, y0:y1, x0:x1r], src,
                                                            wcol(ti), None,
                                                            op0=mybir.AluOpType.mult)
                                eng = nc.gpsimd if mode == 'gps' else nc.vector
                                eng.tensor_add(accv[:, y0:y1, x0:x1r],
                                               accv[:, y0:y1, x0:x1r],
                                               tmpv[:, y0:y1, x0:x1r])
                            nc.vector.tensor_scalar(h2[:], acc[:], ppc(COL_BDW + h), 0.0,
                                                    op0=mybir.AluOpType.add,
                                                    op1=mybir.AluOpType.max)
                        h2_sb.append(h2)

                    # ---- C2: MLP down + residual ----
                    for m in range(NC_C):
                        py = ps_big.tile([128, N], F32, tag="big")
                        for n2 in range(NCHUNK):
                            for h in range(NC_H):
                                nc.tensor.matmul(
                                    py[:, 512 * n2:512 * (n2 + 1)],
                                    lhsT=w2_sb[:, h, 128 * m:128 * (m + 1)],
                                    rhs=h2_sb[h][:, 512 * n2:512 * (n2 + 1)],
                                    start=(h == 0), stop=(h == NC_H - 1))
                        y_sb = yp.tile([128, N], F32, tag="y")
                        nc.vector.scalar_tensor_tensor(
                            y_sb[:], py[:], ppc(COL_B2 + m), x1[:, m, :],
                            op0=mybir.AluOpType.add, op1=mybir.AluOpType.add)
                        nc.sync.dma_start(out=y_e[i, 128 * m:128 * (m + 1), :], in_=y_sb[:])

            if iters == 1:
                body()
            else:
                with tc.For_i(0, iters, 1) as iv:
                    body(iv)
    _split_waits(nc)
    return nc


def _split_waits(nc, max_waits=1):
    """walrus here rejects >1 sync-wait per instruction; hoist extras onto
    same-engine NOPs placed immediately before."""
    for fn in nc.m.functions:
        for bb in fn.blocks:
            out, changed = [], False
            for inst in list(bb.instructions):
                si = inst.sync_info
                if si is not None and len(si.on_wait) > max_waits:
                    waits = list(si.on_wait)
                    for w in waits[:-max_waits]:
                        nop = mybir.InstNoOp(
                            name=nc.get_next_instruction_name(), ins=[], outs=[])
                        nop.engine = inst.engine
                        nop.sync_info = bass_rust.SyncInfo(on_wait=[w], on_update=[])
                        try:
                            nc.register_instruction(nop, overwrite=True)
                        except Exception:
                            pass
                        out.append(nop)
                    inst.sync_info = bass_rust.SyncInfo(
                        on_wait=waits[-max_waits:], on_update=list(si.on_update))
                    changed = True
                out.append(inst)
            if changed:
                bb.instructions = out


def _host_prep(inputs):
    """Fold BN scales into weights, build packed constant tensors (host-side)."""
    f = lambda a: np.asarray(a, dtype=np.float32)
    wq, sq, bq = f(inputs['wq']), f(inputs['sq']), f(inputs['bq'])
    wkv, skv, bkv = f(inputs['wkv']), f(inputs['skv']), f(inputs['bkv'])
    wp_, sp_, bp_ = f(inputs['wp']), f(inputs['sp']), f(inputs['bp'])
    w1, s1, b1 = f(inputs['w1']), f(inputs['s1']), f(inputs['b1'])
    wdw, bdw = f(inputs['wdw']), f(inputs['bdw'])
    w2, s2, b2 = f(inputs['w2']), f(inputs['s2']), f(inputs['b2'])

    bf = lambda a: np.ascontiguousarray(a, dtype=BF16_NP)
    wqT = bf((wq * sq[:, None]).T)          # [384, 256]
    wkvT = bf((wkv * skv[:, None]).T)       # [384, 32]
    wpT = bf((wp_ * sp_[:, None]).T)        # [256, 384]
    w1T = bf((w1 * s1[:, None]).T)          # [384, 1536]
    w2T = bf((w2 * s2[:, None]).T)          # [1536, 384]

    wtap = wdw[:, 0]                        # [1536, 3, 3]; w[c, dy+1, dx+1]

    pp = np.zeros((128, NCOL), np.float32)
    pp[:, COL_BQ + 0] = bq[0:128]
    pp[:, COL_BQ + 1] = bq[128:256]
    pp[:, COL_BKV] = np.tile(bkv, 4)
    for m in range(NC_C):
        pp[:, COL_BP + m] = bp_[128 * m:128 * (m + 1)]
        pp[:, COL_B2 + m] = b2[128 * m:128 * (m + 1)]
    for h in range(NC_H):
        pp[:, COL_B1 + h] = b1[128 * h:128 * (h + 1)]
        pp[:, COL_BDW + h] = bdw[128 * h:128 * (h + 1)]
        for ti, (dy, dx) in enumerate(TAPS):
            pp[:, COL_WT + 9 * h + ti] = wtap[128 * h:128 * (h + 1), dy + 1, dx + 1]
            pp[:, COL_WTN + 9 * h + ti] = -wtap[128 * h:128 * (h + 1), dy + 1, dx + 1]

    sel = np.zeros((2, 128, 8), np.float32)
    for t in range(2):
        for g in range(4):
            sel[t, 32 * g:32 * (g + 1), 4 * t + g] = 1.0
    selb = np.zeros((2, 8, 128), np.float32)
    for t in range(2):
        for g in range(4):
            selb[t, 4 * t + g, 32 * g:32 * (g + 1)] = 1.0
    ident = np.eye(128, dtype=np.float32)

    diag = np.zeros((max(NDIAG, 1), 128, 128), np.float32)
    for h in PE_TILES:
        for ti, (dy, dx) in enumerate(TAPS):
            np.fill_diagonal(diag[9 * DIAG_IDX[h] + ti], wtap[128 * h:128 * (h + 1), dy + 1, dx + 1])

    consts = dict(wqT=wqT, wkvT=wkvT, wpT=wpT, w1T=w1T, w2T=w2T,
                  pp=pp, sel=bf(sel), selb=bf(selb), ident=bf(ident))
    if NDIAG:
        consts['diag'] = bf(diag)
    return consts


_NC_CACHE = {}


def _get_nc(iters=1):
    if iters not in _NC_CACHE:
        _NC_CACHE[iters] = build_nc(iters)
    return _NC_CACHE[iters]


def run_on_hw(inputs, iters=1):
    consts = _host_prep(inputs)
    x = np.asarray(inputs['x'], dtype=np.float32).reshape(B, C, N).astype(BF16_NP)
    in_maps = []
    for core in range(CORES):
        m = dict(consts)
        m['x'] = np.ascontiguousarray(x[BPC * core:BPC * (core + 1)])
        in_maps.append(m)
    nc = _get_nc(iters)
    res = run_bass_kernel_spmd(nc, in_maps, list(range(CORES)))
    y = np.concatenate([res.results[c]['y'] for c in range(CORES)], axis=0)
    return y.reshape(B, C, H, W).astype(np.float32)


def kernel(**inputs):
    return run_on_hw(inputs, iters=1)
